# revision 1
# baseline (speedup 1.0000x reference)
"""GCN layer (GCNConv + BatchNorm1d + ReLU + residual) on 8 Trainium2 cores.

Strategy:
  - Nodes sharded 8 ways (6250/core); edges partitioned by destination core.
  - h[c] = dinv[c] * (sum_{e: dst=c} x'[src[e]]) @ W.T   with x' = x * dinv[:,None]
    (the linear transform commutes with the scatter-add, so W is applied after
    aggregation; the +b term cancels exactly under batch-norm mean subtraction).
  - Per core: compute x' (full copy, redundant across cores), materialize in
    DRAM; dma_gather x'[src] rows (512B each) per destination superblock;
    segmented scatter-add via selection-matrix matmuls accumulating in PSUM
    (S[e,d] = (colrel[e]==d), built with one DVE is_equal against an iota tile);
    per dest block: aggT @ W.T, scale by dinv[dst]; BN batch stats via
    ones-vector matmuls + one 8-core AllReduce of [2,128]; finalize
    out = x + relu(h*s + t).
  - Host side does only index/sharding prep (partition, sort, pad, int16 pack).
"""

import sys

sys.path.insert(0, "/opt/trn_rl_repo")

import numpy as np

import concourse.bacc as bacc
import concourse.mybir as mybir
import concourse.tile as tile
from concourse.bass_utils import run_bass_kernel_spmd
from concourse.masks import make_identity

P = 128
D = 128
F32 = mybir.dt.float32
I32 = mybir.dt.int32
I16 = mybir.dt.int16
BN_EPS = 1e-5
CORES = 8
SBW = 5  # dest blocks per superblock (psum: 5 agg + fin/bc + sh + sh2 = 8 banks)
PH1_W = 1024  # phase-1/3 chunk width (nodes per DMA block)


# ---------------------------------------------------------------- host prep
def _build_plan(x, edge_index, n_nodes):
    N = n_nodes
    npc = N // CORES
    nblk = (npc + P - 1) // P
    npad_local = nblk * P
    NPAD = ((N + P - 1) // P) * P
    GRP = ((NPAD // 2 + P - 1) // P) * P  # source-group width (int16 range)
    assert GRP <= 32767 and NPAD - GRP <= 32767

    src = np.asarray(edge_index[0]).astype(np.int64).astype(np.int32)
    dst = np.asarray(edge_index[1]).astype(np.int64).astype(np.int32)
    # self-loops are NOT added to the edge streams: their contribution
    # (x'[c] into agg[c]) is added on-device via a PE transpose-accumulate.
    # They still count toward the degree.
    deg = (np.bincount(dst, minlength=N) + 1).astype(np.float32)

    core_of = dst // npc
    dloc = dst - core_of * npc
    db_l = dloc // P
    g_l = (src >= GRP).astype(np.int32)
    sb_l = db_l // SBW
    nsb = (nblk + SBW - 1) // SBW

    order = np.lexsort((db_l, g_l, sb_l, core_of))
    src_s, dloc_s = src[order], dloc[order]
    core_s, db_s, g_s = core_of[order], db_l[order], g_l[order]

    cnt = np.zeros((CORES, nblk, 2), np.int64)
    np.add.at(cnt, (core_s, db_s, g_s), 1)
    T = ((cnt.max(axis=0) + P - 1) // P).astype(np.int64)  # [nblk, 2] tiles

    # cumulative offsets in processing order (core, sb, g, db)
    offs = np.zeros((CORES, nsb, 2, SBW), np.int64)
    run = 0
    for c in range(CORES):
        for sb in range(nsb):
            for g in range(2):
                for j in range(SBW):
                    db = sb * SBW + j
                    if db >= nblk:
                        continue
                    offs[c, sb, g, j] = run
                    run += cnt[c, db, g]
    total_real = run
    assert total_real == src.shape[0]

    tiles_total = int(T.sum())
    tot_e = tiles_total * P
    s_tot = tot_e // 16

    idx_streams = np.zeros((CORES, tot_e), np.int16)
    cr_streams = np.full((CORES, tot_e), -1.0, np.float32)
    n_idx_sb = np.zeros((nsb, 2), np.int64)
    pos = 0
    tpos = 0
    slot_pos = np.zeros((nsb, 2, SBW), np.int64)  # edge offset of slot in stream
    for sb in range(nsb):
        for g in range(2):
            for j in range(SBW):
                db = sb * SBW + j
                if db >= nblk:
                    continue
                slot_pos[sb, g, j] = pos
                w = int(T[db, g]) * P
                n_idx_sb[sb, g] += w
                pos += w
    assert pos == tot_e

    for c in range(CORES):
        for sb in range(nsb):
            for g in range(2):
                for j in range(SBW):
                    db = sb * SBW + j
                    if db >= nblk:
                        continue
                    k = int(cnt[c, db, g])
                    if k == 0:
                        continue
                    o = int(offs[c, sb, g, j])
                    p0 = int(slot_pos[sb, g, j])
                    idx_streams[c, p0 : p0 + k] = (
                        src_s[o : o + k] - g * GRP
                    ).astype(np.int16)
                    cr_streams[c, p0 : p0 + k] = (
                        dloc_s[o : o + k] - db * P
                    ).astype(np.float32)

    # pack: idx16 [128, s_tot] replicated over 8 groups of 16 partitions;
    # colrel [128, tiles_total]
    idx16 = np.zeros((CORES, P, s_tot), np.int16)
    colrel = np.zeros((CORES, P, tiles_total), np.float32)
    for c in range(CORES):
        idx16[c] = np.tile(idx_streams[c].reshape(-1, 16).T, (8, 1))
        colrel[c] = cr_streams[c].reshape(-1, P).T

    deg_pad = np.concatenate([deg, np.ones(NPAD - N, np.float32)])
    # packed to match phase-1 "(p a) k" tiles: col b*A+a holds
    # deg[b*1024 + p*A + a] at partition p (A = rows per partition)
    PH1_W_ = 1024
    cols = []
    for b0 in range(0, NPAD, PH1_W_):
        w = min(PH1_W_, NPAD - b0)
        cols.append(deg_pad[b0 : b0 + w].reshape(P, w // P))
    deg_pm = np.concatenate(cols, axis=1)  # [128, NPAD/128]

    deg_own = np.zeros((CORES, P, nblk), np.float32)
    xres = np.zeros((CORES, npad_local, D), np.float32)
    for c in range(CORES):
        dg = np.ones(npad_local, np.float32)
        dg[:npc] = deg[c * npc : (c + 1) * npc]
        deg_own[c] = dg.reshape(nblk, P).T
        xres[c, :npc] = x[c * npc : (c + 1) * npc]

    x_pad = np.zeros((NPAD, D), np.float32)
    x_pad[:N] = x

    return dict(
        N=N, npc=npc, nblk=nblk, npad_local=npad_local, NPAD=NPAD, GRP=GRP,
        nsb=nsb, T=T, n_idx_sb=n_idx_sb, tiles_total=tiles_total,
        s_tot=s_tot, idx16=idx16, colrel=colrel, deg_pm=deg_pm,
        deg_own=deg_own, xres=xres, x_pad=x_pad,
    )


# ------------------------------------------------------------- device build
def _build_program(plan, W, gamma, beta):
    N = plan["N"]
    nblk, nsb = plan["nblk"], plan["nsb"]
    NPAD, GRP = plan["NPAD"], plan["GRP"]
    npc, npad_local = plan["npc"], plan["npad_local"]
    T = plan["T"]
    n_idx_sb = plan["n_idx_sb"]
    tiles_total, s_tot = plan["tiles_total"], plan["s_tot"]
    ncols = NPAD // P

    nc = bacc.Bacc("TRN2", target_bir_lowering=False, debug=False,
                   num_devices=CORES)

    x_d = nc.declare_dram_parameter("x", [NPAD, D], F32, isOutput=False)
    xres_d = nc.declare_dram_parameter("xres", [npad_local, D], F32,
                                       isOutput=False)
    degpm_d = nc.declare_dram_parameter("degpm", [P, ncols], F32,
                                        isOutput=False)
    degown_d = nc.declare_dram_parameter("degown", [P, nblk], F32,
                                         isOutput=False)
    W_d = nc.declare_dram_parameter("W", [D, D], F32, isOutput=False)
    gamma_d = nc.declare_dram_parameter("gamma", [1, D], F32, isOutput=False)
    beta_d = nc.declare_dram_parameter("beta", [1, D], F32, isOutput=False)
    idx_d = nc.declare_dram_parameter("idx16", [P, s_tot], I16, isOutput=False)
    cr_d = nc.declare_dram_parameter("colrel", [P, tiles_total], F32,
                                     isOutput=False)
    out_d = nc.declare_dram_parameter("out", [npc, D], F32, isOutput=True)

    xp_d = nc.dram_tensor("xp", [NPAD, D], F32)
    cc_in = nc.dram_tensor("cc_in", [2, D], F32)
    cc_out = nc.dram_tensor("cc_out", [2, D], F32, addr_space="Shared")

    with tile.TileContext(nc) as tc:
        with tc.tile_pool(name="const", bufs=1) as cpool, \
             tc.tile_pool(name="work", bufs=3) as wpool, \
             tc.tile_pool(name="gath", bufs=2) as gpool, \
             tc.tile_pool(name="psum", bufs=1, space="PSUM") as ppool:

            # ---- constants
            iota_i = cpool.tile([P, P], I32)
            nc.gpsimd.iota(iota_i[:], pattern=[[1, P]], base=0,
                           channel_multiplier=0)
            iota_f = cpool.tile([P, P], F32)
            nc.vector.tensor_copy(iota_f[:], iota_i[:])

            ident = cpool.tile([P, P], F32)
            make_identity(nc, ident[:])

            w_sb = cpool.tile([D, D], F32)
            nc.sync.dma_start(out=w_sb[:], in_=W_d[:, :])
            wt_ps = ppool.tile([D, D], F32, tag="fin")
            nc.tensor.transpose(out=wt_ps[:], in_=w_sb[:], identity=ident[:])
            w_t = cpool.tile([D, D], F32)
            nc.vector.tensor_copy(w_t[:], wt_ps[:])

            degpm_sb = cpool.tile([P, ncols], F32)
            nc.sync.dma_start(out=degpm_sb[:], in_=degpm_d[:, :])
            sq_all = cpool.tile([P, ncols], F32)
            nc.scalar.activation(sq_all[:], degpm_sb[:],
                                 mybir.ActivationFunctionType.Sqrt)
            dinv_all = cpool.tile([P, ncols], F32)
            nc.vector.reciprocal(dinv_all[:], sq_all[:])

            degown_sb = cpool.tile([P, nblk], F32)
            nc.sync.dma_start(out=degown_sb[:], in_=degown_d[:, :])
            sq_own = cpool.tile([P, nblk], F32)
            nc.scalar.activation(sq_own[:], degown_sb[:],
                                 mybir.ActivationFunctionType.Sqrt)
            dinv_own = cpool.tile([P, nblk], F32)
            nc.vector.reciprocal(dinv_own[:], sq_own[:])

            gamma_sb = cpool.tile([1, D], F32)
            nc.sync.dma_start(out=gamma_sb[:], in_=gamma_d[:, :])
            beta_sb = cpool.tile([1, D], F32)
            nc.sync.dma_start(out=beta_sb[:], in_=beta_d[:, :])

            ones_col = cpool.tile([P, 1], F32)
            nc.vector.memset(ones_col[:], 1.0)
            ones_row = cpool.tile([1, P], F32)
            nc.vector.memset(ones_row[:], 1.0)

            idx_sb = cpool.tile([P, s_tot], I16)
            nc.sync.dma_start(out=idx_sb[:], in_=idx_d[:, :])
            cr_sb = cpool.tile([P, tiles_total], F32)
            nc.sync.dma_start(out=cr_sb[:], in_=cr_d[:, :])

            h_buf = cpool.tile([P, nblk * P], F32)

            xres_sb = cpool.tile([P, nblk * P], F32)
            nc.sync.dma_start(
                out=xres_sb[:].rearrange("p (a k) -> p a k", k=P),
                in_=xres_d[:, :].rearrange("(a p) k -> p a k", p=P))

            # ---- phase 1: x' = x * dinv  -> xp_d
            # "(p a) k" packing: partition p holds PH1_A consecutive rows,
            # giving 4KB-contiguous DMA descriptors per partition.
            import os as _os
            _PH = int(_os.environ.get("KPH", "4"))
            _NOMM = _os.environ.get("KNOMM", "") == "1"
            _NOSEL = _os.environ.get("KNOSEL", "") == "1"
            PH1_A = PH1_W // P  # rows per partition per block
            nb1 = (NPAD + PH1_W - 1) // PH1_W
            for b in range(nb1):
                w = min(PH1_W, NPAD - b * PH1_W)
                nch = w // P
                xt = wpool.tile([P, w], F32, tag="xt")
                nc.sync.dma_start(
                    out=xt[:].rearrange("p (a k) -> p a k", k=P),
                    in_=x_d[b * PH1_W : b * PH1_W + w, :].rearrange(
                        "(p a) k -> p a k", a=nch))
                for a in range(nch):
                    col = b * PH1_A + a
                    nc.vector.tensor_scalar_mul(
                        xt[:, a * P : (a + 1) * P],
                        xt[:, a * P : (a + 1) * P],
                        dinv_all[:, col : col + 1])
                nc.sync.dma_start(
                    out=xp_d[b * PH1_W : b * PH1_W + w, :].rearrange(
                        "(p a) k -> p a k", a=nch),
                    in_=xt[:].rearrange("p (a k) -> p a k", k=P))

            # ---- phase 2: gather + selection-matmul aggregation
            if _PH < 2 or _NOMM:
                nc.vector.memset(h_buf[:], 0.0)

            sh_ps = ppool.tile([1, D], F32, tag="sh", name="sh_ps")
            sh2_ps = ppool.tile([1, D], F32, tag="sh2", name="sh2_ps")
            agg_buf = cpool.tile([P, nblk * P], F32)

            GCH = 1024  # max idxs per dma_gather (HW packet limit)
            seg_tile0 = np.zeros((nsb, 2), np.int64)
            seg_idx0 = np.zeros((nsb, 2), np.int64)
            tcur = 0
            for sb in range(nsb):
                for g in range(2):
                    seg_tile0[sb, g] = tcur
                    seg_idx0[sb, g] = tcur * (P // 16)
                    tcur += n_idx_sb[sb, g] // P

            def emit_gather(sb, g):
                n_idx = int(n_idx_sb[sb, g])
                gt = gpool.tile([P, n_idx], F32, tag="gt",
                                name=f"gt_{sb}_{g}")
                c0 = int(seg_idx0[sb, g])
                for k0 in range(0, n_idx, GCH):
                    n = min(GCH, n_idx - k0)
                    nc.gpsimd.dma_gather(
                        out_ap=gt[:, k0 : k0 + n].rearrange(
                            "p (t k) -> p t k", k=P),
                        in_ap=xp_d[g * GRP : min((g + 1) * GRP, NPAD), :],
                        idxs_ap=idx_sb[:, c0 + k0 // 16 : c0 + (k0 + n) // 16],
                        num_idxs=n, num_idxs_reg=n, elem_size=D)
                return gt

            def emit_mms(sb, g, gt, psums, blks, is_first_pass):
                loc = 0
                for j, db in enumerate(blks):
                    first = True
                    for t in range(int(T[db, g])):
                        gtile = int(seg_tile0[sb, g]) + loc
                        s_t = wpool.tile([P, P], F32, tag="s_t",
                                         name=f"s_{sb}_{g}_{loc}")
                        nc.vector.tensor_tensor(
                            out=s_t[:], in0=iota_f[:],
                            in1=cr_sb[:, gtile : gtile + 1].to_broadcast(
                                [P, P]),
                            op=mybir.AluOpType.is_equal)
                        nc.tensor.matmul(
                            out=psums[db][:],
                            lhsT=gt[:, loc * P : (loc + 1) * P],
                            rhs=s_t[:],
                            start=(first and not is_first_pass),
                            stop=(t == int(T[db, g]) - 1))
                        first = False
                        loc += 1

            # ---- pass A: source group 0 (+ self-loop transpose)
            for sb in range(nsb if _PH >= 2 else 0):
                blks = list(range(sb * SBW, min((sb + 1) * SBW, nblk)))
                gt = emit_gather(sb, 0) if n_idx_sb[sb, 0] else None
                psums = {}
                for j, db in enumerate(blks):
                    psums[db] = ppool.tile([P, P], F32, tag=f"agg{j}",
                                           name=f"aggA_{db}")
                    # x'_own = xres * dinv (self-loop term)
                    xo = wpool.tile([P, P], F32, tag="xo", name=f"xo_{db}")
                    nc.vector.tensor_scalar_mul(
                        xo[:], xres_sb[:, db * P : (db + 1) * P],
                        dinv_own[:, db : db + 1])
                    nc.tensor.matmul(
                        out=psums[db][:], lhsT=xo[:], rhs=ident[:],
                        is_transpose=True, start=True,
                        stop=(int(T[db, 0]) == 0))
                if gt is not None:
                    emit_mms(sb, 0, gt, psums, blks, True)
                for j, db in enumerate(blks):
                    nc.vector.tensor_copy(
                        agg_buf[:, db * P : (db + 1) * P], psums[db][:])

            # ---- pass B: source group 1, then finalize + BN stats
            for sb in range(nsb if _PH >= 2 else 0):
                blks = list(range(sb * SBW, min((sb + 1) * SBW, nblk)))
                gt = emit_gather(sb, 1) if n_idx_sb[sb, 1] else None
                psums = {}
                for j, db in enumerate(blks):
                    if int(T[db, 1]) > 0:
                        psums[db] = ppool.tile([P, P], F32, tag=f"agg{j}",
                                               name=f"aggB_{db}")
                if gt is not None:
                    emit_mms(sb, 1, gt, psums, blks, False)
                for j, db in enumerate(blks if not _NOMM else []):
                    aggt = wpool.tile([P, P], F32, tag="aggt",
                                      name=f"aggt_{db}")
                    ab = agg_buf[:, db * P : (db + 1) * P]
                    if db in psums:
                        nc.vector.tensor_tensor(out=aggt[:], in0=ab,
                                                in1=psums[db][:],
                                                op=mybir.AluOpType.add)
                    else:
                        nc.vector.tensor_copy(aggt[:], ab)
                    fin = ppool.tile([P, P], F32, tag="fin",
                                     name=f"fin_{db}")
                    nc.tensor.matmul(out=fin[:], lhsT=aggt[:], rhs=w_t[:],
                                     start=True, stop=True)
                    hb = h_buf[:, db * P : (db + 1) * P]
                    nc.vector.tensor_scalar_mul(
                        hb, fin[:], dinv_own[:, db : db + 1])
                    if _PH >= 2 and not _NOMM:
                        sq = wpool.tile([P, P], F32, tag="s_t",
                                        name=f"sq_{db}")
                        nc.vector.tensor_tensor(out=sq[:], in0=hb, in1=hb,
                                                op=mybir.AluOpType.mult)
                        nc.tensor.matmul(out=sh_ps[:], lhsT=ones_col[:],
                                         rhs=hb, start=(db == 0),
                                         stop=(db == nblk - 1))
                        nc.tensor.matmul(out=sh2_ps[:], lhsT=ones_col[:],
                                         rhs=sq[:], start=(db == 0),
                                         stop=(db == nblk - 1))

            # ---- phase 2b: BN stats reduce + normalize constants
            _dummy_stats = _PH < 3
            if _PH < 2 or _NOMM:
                nc.tensor.matmul(out=sh_ps[:], lhsT=ones_col[:],
                                 rhs=h_buf[:, :D], start=True, stop=True)
                nc.tensor.matmul(out=sh2_ps[:], lhsT=ones_col[:],
                                 rhs=h_buf[:, :D], start=True, stop=True)
            sh_sb = cpool.tile([1, D], F32)
            nc.vector.tensor_copy(sh_sb[:], sh_ps[:])
            sh2_sb = cpool.tile([1, D], F32)
            nc.vector.tensor_copy(sh2_sb[:], sh2_ps[:])
            if not _dummy_stats:
                nc.sync.dma_start(out=cc_in[0:1, :], in_=sh_sb[:])
                nc.sync.dma_start(out=cc_in[1:2, :], in_=sh2_sb[:])
                nc.gpsimd.collective_compute(
                    "AllReduce", mybir.AluOpType.add,
                    ins=[cc_in[:]], outs=[cc_out[:]],
                    replica_groups=[list(range(CORES))])
            gsum = cpool.tile([1, D], F32)
            gsum2 = cpool.tile([1, D], F32)
            if _dummy_stats:
                nc.vector.memset(gsum[:], 1.0)
                nc.vector.memset(gsum2[:], 2.0)
            else:
                nc.sync.dma_start(out=gsum[:], in_=cc_out[0:1, :])
                nc.sync.dma_start(out=gsum2[:], in_=cc_out[1:2, :])

            mean = cpool.tile([1, D], F32)
            nc.vector.tensor_scalar_mul(mean[:], gsum[:], 1.0 / N)
            eh2 = cpool.tile([1, D], F32)
            nc.vector.tensor_scalar_mul(eh2[:], gsum2[:], 1.0 / N)
            msq = cpool.tile([1, D], F32)
            nc.vector.tensor_tensor(out=msq[:], in0=mean[:], in1=mean[:],
                                    op=mybir.AluOpType.mult)
            var = cpool.tile([1, D], F32)
            nc.vector.tensor_tensor(out=var[:], in0=eh2[:], in1=msq[:],
                                    op=mybir.AluOpType.subtract)
            vare = cpool.tile([1, D], F32)
            nc.vector.tensor_scalar_add(vare[:], var[:], BN_EPS)
            sdev = cpool.tile([1, D], F32)
            nc.scalar.activation(sdev[:], vare[:],
                                 mybir.ActivationFunctionType.Sqrt)
            rstd = cpool.tile([1, D], F32)
            nc.vector.reciprocal(rstd[:], sdev[:])

            st_row = cpool.tile([1, 2 * D], F32)
            nc.vector.tensor_tensor(out=st_row[:, :D], in0=rstd[:],
                                    in1=gamma_sb[:], op=mybir.AluOpType.mult)
            # t = beta - mean * s
            ms = cpool.tile([1, D], F32)
            nc.vector.tensor_tensor(out=ms[:], in0=mean[:],
                                    in1=st_row[:, :D],
                                    op=mybir.AluOpType.mult)
            nc.vector.tensor_tensor(out=st_row[:, D:], in0=beta_sb[:],
                                    in1=ms[:], op=mybir.AluOpType.subtract)
            bc_ps = ppool.tile([P, 2 * D], F32, tag="fin")
            nc.tensor.matmul(out=bc_ps[:], lhsT=ones_row[:], rhs=st_row[:],
                             start=True, stop=True)
            s_rep = cpool.tile([P, D], F32)
            nc.vector.tensor_copy(s_rep[:], bc_ps[:, :D])
            t_rep = cpool.tile([P, D], F32)
            nc.vector.tensor_copy(t_rep[:], bc_ps[:, D:])

            # ---- phase 3: out = xres + relu(h*s + t)
            nb3 = (npad_local + PH1_W - 1) // PH1_W
            for b in range(nb3 if _PH >= 4 else 0):
                w = min(PH1_W, npad_local - b * PH1_W)
                nch = w // P
                ot = wpool.tile([P, w], F32, tag="ot")
                for a in range(nch):
                    db = b * (PH1_W // P) + a
                    hb = h_buf[:, db * P : (db + 1) * P]
                    tmp = wpool.tile([P, P], F32, tag="s_t")
                    nc.vector.tensor_tensor(out=tmp[:], in0=hb, in1=s_rep[:],
                                            op=mybir.AluOpType.mult)
                    nc.vector.tensor_tensor(out=tmp[:], in0=tmp[:],
                                            in1=t_rep[:],
                                            op=mybir.AluOpType.add)
                    osl = ot[:, a * P : (a + 1) * P]
                    nc.scalar.activation(osl, tmp[:],
                                         mybir.ActivationFunctionType.Relu)
                    nc.vector.tensor_tensor(
                        out=osl, in0=osl,
                        in1=xres_sb[:, db * P : (db + 1) * P],
                        op=mybir.AluOpType.add)
                # store valid rows only
                lo = b * PH1_W
                hi = min(npc, lo + w)
                if hi <= lo:
                    continue
                nv = hi - lo
                full = nv // P
                if full > 0:
                    nc.sync.dma_start(
                        out=out_d[lo : lo + full * P, :].rearrange(
                            "(a p) k -> p a k", p=P),
                        in_=ot[:, : full * P].rearrange(
                            "p (a k) -> p a k", k=P))
                rem = nv - full * P
                if rem > 0:
                    nc.sync.dma_start(
                        out=out_d[lo + full * P : hi, :],
                        in_=ot[:rem, full * P : (full + 1) * P])

    nc.compile()
    return nc


# ------------------------------------------------------------------ driver
_CACHE = {}
TRACE = False        # set True (e.g. from a bench harness) to profile
RUN_KWARGS = None
LAST_RESULT = None


def kernel(**inputs):
    x = np.asarray(inputs["x"], np.float32)
    edge_index = np.asarray(inputs["edge_index"])
    W = np.asarray(inputs["W"], np.float32)
    gamma = np.asarray(inputs["gamma"], np.float32)
    beta = np.asarray(inputs["beta"], np.float32)
    # inputs["b"] shifts h uniformly and cancels under batch-norm mean
    # subtraction, so it does not affect the output.
    N = x.shape[0]

    plan = _build_plan(x, edge_index, N)
    key = (N, edge_index.shape[1], plan["tiles_total"],
           tuple(plan["T"].ravel().tolist()))
    if key not in _CACHE:
        _CACHE[key] = _build_program(plan, W, gamma, beta)
    nc = _CACHE[key]

    in_maps = []
    for c in range(CORES):
        in_maps.append({
            "x": plan["x_pad"],
            "xres": plan["xres"][c],
            "degpm": plan["deg_pm"],
            "degown": plan["deg_own"][c],
            "W": W,
            "gamma": gamma.reshape(1, -1),
            "beta": beta.reshape(1, -1),
            "idx16": plan["idx16"][c],
            "colrel": plan["colrel"][c],
        })

    res = run_bass_kernel_spmd(nc, in_maps, list(range(CORES)),
                               trace=TRACE, **(RUN_KWARGS or {}))
    global LAST_RESULT
    LAST_RESULT = res
    out = np.concatenate([res.results[c]["out"] for c in range(CORES)],
                         axis=0)
    return out.astype(np.float32)

